# revision 1
# baseline (speedup 1.0000x reference)
"""Multi-head self-attention (B=4, S=2048, D=512, H=8, d=64) on 8 trn2 cores.

Sharding: 2 cores per batch element; each core computes 4 heads (a 256-wide
column slice of Wq/Wk/Wv and row slice of Wh) and produces a partial
[S, 512] output; the host sums the two partials per batch and adds bh.

Per-core pipeline (matmul operands float32r — fp32 data streamed at bf16
rate with ~13-bit-mantissa input rounding; x^T is pre-transposed on the
host and DMAed straight into an f32r tile, so nothing is staged or cast):
  A) project qT/kT [256, S] and v [S, 256] from the resident x^T
     (v stored augmented with a ones column per head so the attention
     matmul also produces the softmax denominator row).
  B) heads processed in pairs packed on PE row-groups (K=64 each at base
     partitions 0/64): scores^T -> one FD-1024 exp on ScalarE covering
     both heads (no max-subtraction — logits are O(5)); the augmented-V
     matmul accumulates attnU^T[d,sq] + the Z row; normalize via DVE
     reciprocal + K=1 ones-matmul broadcast + tensor_tensor multiply.
  C) out[s,:] = attnT.T @ Wh accumulated over the 256 head dims,
     emitted one quarter behind attention so PE's in-order queue never
     delays the next quarter's scores.

ScalarE's exp stream (~133us busy at ~97% density) is the bottleneck and
shadows all matmul/DMA/normalization work; measured ~245us end-to-end.
"""

import numpy as np

NUM_HEADS = 8
D_MODEL = 512
D_HEAD = 64
B = 4
S = 2048
H_PER_CORE = 4          # heads per core
DQ = H_PER_CORE * D_HEAD  # 256 = per-core q/k/v width
N_CORES = 8
SCALE = 1.0 / np.sqrt(D_HEAD)

_KO = D_MODEL // 128    # 4 contraction chunks for the projections
_NT = S // 128          # 16 tiles of 128 along S
_VW = D_HEAD + 1        # 65: v columns per head incl. ones column


def _split_excess_waits(nc):
    """Walrus's TRN2 codegen fits very few sync-waits per instruction (one on
    a Matmult's weight-load, few on drains).  Move excess waits onto NoOps
    inserted just before the instruction — engine queues are in-order, so a
    wait on a preceding same-engine instruction still protects it."""
    import concourse.mybir as mybir

    n_fixed = 0
    for f in nc.m.functions:
        for bb in f.blocks:
            insts = list(bb.instructions)
            out = []
            changed = False
            for ins in insts:
                si = ins.sync_info
                if si is not None and si.on_wait and len(si.on_wait) > 1:
                    waits = list(si.on_wait)
                    # An exp/matmul waiting on its OWN engine's completion sem
                    # is a slot-recycle WAW guard: implied by in-order issue,
                    # with the interleaved cross-engine reader guarded by the
                    # remaining wait.  Dropping it avoids a NoOp on the
                    # bottleneck queue (one per exp otherwise).
                    if isinstance(ins, (mybir.InstActivation, mybir.InstMatmult)):
                        eng_pfx = str(ins.engine).split(".")[-1] + "_"
                        cross = [w for w in waits
                                 if not str(getattr(w, "ant_name", "")).startswith(eng_pfx)]
                        if cross and len(cross) < len(waits):
                            waits = cross
                    for j, w in enumerate(waits[1:]):
                        nop = mybir.InstNoOp(
                            name=f"{ins.name}_waitnop{j}", ins=[], outs=[])
                        nop.engine = ins.engine
                        nop.sync_info = mybir.SyncInfo(on_wait=[w], on_update=[])
                        out.append(nop)
                    ins.sync_info = mybir.SyncInfo(
                        on_wait=waits[:1], on_update=list(si.on_update or []))
                    n_fixed += 1
                    changed = True
                out.append(ins)
            if changed:
                bb.instructions = out
    return n_fixed


def build_nc(nrep=1):
    """Build the per-core Bass program.  nrep>1 repeats the compute body
    (same tiles, idempotent) for wall-clock timing amplification."""
    import concourse.bass as bass
    import concourse.mybir as mybir
    import concourse.tile as tile
    from concourse.masks import make_identity

    f32 = mybir.dt.float32
    f32r = mybir.dt.float32r
    AF = mybir.ActivationFunctionType

    nc = bass.Bass()
    x_d = nc.dram_tensor("x", [D_MODEL, S], f32r, kind="ExternalInput")
    wq_d = nc.dram_tensor("wq", [D_MODEL, DQ], f32, kind="ExternalInput")
    wk_d = nc.dram_tensor("wk", [D_MODEL, DQ], f32, kind="ExternalInput")
    wv_d = nc.dram_tensor("wv", [D_MODEL, DQ], f32, kind="ExternalInput")
    wh_d = nc.dram_tensor("wh", [DQ, D_MODEL], f32, kind="ExternalInput")
    bq_d = nc.dram_tensor("bq", [DQ], f32, kind="ExternalInput")
    bk_d = nc.dram_tensor("bk", [DQ], f32, kind="ExternalInput")
    bv_d = nc.dram_tensor("bv", [DQ], f32, kind="ExternalInput")
    o_d = nc.dram_tensor("o", [S, D_MODEL], f32, kind="ExternalOutput")

    with (
        nc.allow_low_precision(reason="float32r attention pipeline"),
        tile.TileContext(nc) as tc,
        tc.tile_pool(name="cst", bufs=1) as cst,
        tc.tile_pool(name="big", bufs=1) as big,
        tc.tile_pool(name="pr", bufs=4) as pr,
        tc.tile_pool(name="ps_sc", bufs=2, space="PSUM") as ps_sc,
        tc.tile_pool(name="ps_av", bufs=3, space="PSUM") as ps_av,
        tc.tile_pool(name="ps_bc", bufs=1, space="PSUM") as ps_bc,
    ):
        ones_col = cst.tile([1, D_HEAD], f32r)
        nc.gpsimd.memset(ones_col.bitcast(f32)[:], 1.0)

        # ---- load + cast weights/biases (once) ----
        w_sb = {}
        for name, dram, shp in (
            ("wq", wq_d, (128, _KO, DQ)),
            ("wk", wk_d, (128, _KO, DQ)),
            ("wv", wv_d, (128, _KO, DQ)),
            ("wh", wh_d, (128, DQ // 128, D_MODEL)),
        ):
            raw = pr.tile(list(shp), f32, tag="wraw")
            nc.scalar.dma_start(raw[:], dram.rearrange("(a p) m -> p a m", p=128))
            cvt = big.tile(list(shp), f32r, tag=f"w_{name}")
            nc.vector.tensor_copy(cvt[:], raw[:])
            w_sb[name] = cvt
        bias_sb = {}
        for name, dram in (("bq", bq_d), ("bk", bk_d), ("bv", bv_d)):
            bt = cst.tile([128, DQ // 128], f32, tag=f"b_{name}")
            nc.scalar.dma_start(bt[:], dram.rearrange("(o p) -> p o", p=128))
            bias_sb[name] = bt

        # ---- load x^T (host pre-transposed) straight into f32r ----
        xT0 = big.tile([128, _KO, S], f32r, tag="xT")
        x_engs = [nc.sync, nc.gpsimd, nc.scalar, nc.sync,
                  nc.gpsimd, nc.scalar, nc.sync, nc.gpsimd]
        for sg in range(8):
            x_engs[sg].dma_start(
                xT0[:, :, sg * 256:(sg + 1) * 256],
                x_d.rearrange("(a p) s -> p a s", p=128)[:, :, sg * 256:(sg + 1) * 256])

        for _rep in range(nrep):
            xT = xT0
            qT = big.tile([128, DQ // 128, S], f32r, tag="qT")
            kT = big.tile([128, DQ // 128, S], f32r, tag="kT")
            attnT = big.tile([128, DQ // 128, S], f32r, tag="attnT")
            v_aug = big.tile([128, _NT, H_PER_CORE * _VW], f32r, tag="v_aug")
            nc.gpsimd.memset(v_aug.bitcast(f32)[:], 1.0)

            def transpose_group(tg):
                pass  # x^T already resident in f32r

            def proj_group(dst, wname, bname, o, sg, on_act=False):
                p = ps_sc.tile([128, 1024], f32, tag="sc", name="p_qk")
                for ko in range(_KO):
                    nc.tensor.matmul(
                        p[:, :512],
                        w_sb[wname][:, ko, o * 128:(o + 1) * 128],
                        xT[:, ko, sg * 512:(sg + 1) * 512],
                        start=(ko == 0), stop=(ko == _KO - 1))
                if on_act:
                    nc.scalar.activation(
                        dst[:, o, sg * 512:(sg + 1) * 512], p[:, :512],
                        AF.Identity, bias=bias_sb[bname][:, o:o + 1])
                else:
                    nc.vector.tensor_scalar_add(
                        dst[:, o, sg * 512:(sg + 1) * 512], p[:, :512],
                        bias_sb[bname][:, o:o + 1])

            def project_v(t0, t1):
                for t in range(t0, t1):
                    pv = ps_av.tile([128, 512], f32, tag="av", name="pv")
                    for ko in range(_KO):
                        nc.tensor.matmul(
                            pv[:, :DQ],
                            xT[:, ko, t * 128:(t + 1) * 128],
                            w_sb["wv"][:, ko, :],
                            start=(ko == 0), stop=(ko == _KO - 1))
                    nc.vector.tensor_copy(
                        v_aug[:, t, :].rearrange(
                            "p (h w) -> p h w", w=_VW)[:, :, :D_HEAD],
                        pv[:, :DQ].rearrange("p (h w) -> p h w", w=D_HEAD))

            def attend_pair_quarter(o, jq, t0=0, t1=_NT, avs=None):
                # heads (2o, 2o+1) at base partitions 0 / 64, packed on PE
                # row groups; each sc tile is [t=128, h0-sq512 | h1-sq512].
                h0, h1 = 2 * o, 2 * o + 1
                sq = jq * 512
                if avs is None:
                    av0 = ps_av.tile([128, 512], f32, tag="av", name="av0")
                    av1 = ps_av.tile([128, 512], f32, tag="av", name="av1")
                else:
                    av0, av1 = avs
                # Software-pipelined by one t-step: av(t) needs exp(t), so
                # emit sc(t+1) ahead of av(t) — PE computes the next tile's
                # scores while ScalarE exps this one, instead of blocking
                # in-order on the exp result.
                def emit_av(t, probs):
                    nc.tensor.matmul(
                        av0[0:_VW, :],
                        v_aug[:, t, h0 * _VW:(h0 + 1) * _VW],
                        probs[:, 0:512],
                        start=(t == 0), stop=(t == _NT - 1))
                    nc.tensor.matmul(
                        av1[0:_VW, :],
                        v_aug[:, t, h1 * _VW:(h1 + 1) * _VW],
                        probs[:, 512:1024],
                        start=(t == 0), stop=(t == _NT - 1))

                pending = None
                for t in range(t0, t1):
                    sc = ps_sc.tile([128, 1024], f32, tag="sc", name="sc")
                    nc.tensor.matmul(
                        sc[:, 0:512],
                        kT[0:64, o, t * 128:(t + 1) * 128],
                        qT[0:64, o, sq:sq + 512],
                        start=True, stop=True)
                    nc.tensor.matmul(
                        sc[:, 512:1024],
                        kT[64:128, o, t * 128:(t + 1) * 128],
                        qT[64:128, o, sq:sq + 512],
                        start=True, stop=True)
                    probs = pr.tile([128, 1024], f32r, tag="probs")
                    nc.scalar.activation(probs[:], sc[:], AF.Exp,
                                         scale=float(SCALE))
                    if pending is not None:
                        emit_av(*pending)
                    pending = (t, probs)
                emit_av(*pending)
                if t1 < _NT:
                    return (av0, av1)
                for hh, av in ((h0, av0), (h1, av1)):
                    bp = 64 * (hh % 2)
                    invZ = pr.tile([1, 512], f32r, tag="invz")
                    nc.vector.reciprocal(invZ[:], av[D_HEAD:_VW, :])
                    bc = ps_bc.tile([64, 512], f32, tag="bc", name="bc")
                    nc.tensor.matmul(bc[:], ones_col[:], invZ[:],
                                     start=True, stop=True)
                    bc_sb = pr.tile([64, 512], f32, tag="bc_sb")
                    nc.vector.tensor_copy(bc_sb[:], bc[:])
                    nc.vector.tensor_tensor(
                        attnT[bp:bp + 64, o, sq:sq + 512],
                        av[0:D_HEAD, :], bc_sb[:], mybir.AluOpType.mult)

            def out_quarter(jq):
                # out rows [512*jq, 512*jq+512) need attnT for all heads there
                for o in range(DQ // 128):
                    nc.vector.tensor_scalar_add(
                        attnT[:, o, jq * 512:(jq + 1) * 512],
                        attnT[:, o, jq * 512:(jq + 1) * 512],
                        bias_sb["bv"][:, o:o + 1])
                for sg in range(4 * jq, 4 * jq + 4):
                    po = ps_av.tile([128, 512], f32, tag="av", name="po")
                    for o in range(DQ // 128):
                        nc.tensor.matmul(
                            po[:],
                            attnT[:, o, sg * 128:(sg + 1) * 128],
                            w_sb["wh"][:, o, :],
                            start=(o == 0), stop=(o == DQ // 128 - 1))
                    ot = pr.tile([128, 512], f32, tag="ot")
                    nc.vector.tensor_copy(ot[:], po[:])
                    oeng = nc.sync if sg % 2 == 0 else nc.gpsimd
                    oeng.dma_start(
                        o_d.rearrange("(t p) d -> p t d", p=128)[:, sg, :], ot[:])

            transpose_group(0)
            proj_group(kT, "wk", "bk", 0, 0, on_act=True)
            transpose_group(1)
            proj_group(kT, "wk", "bk", 0, 1, on_act=True)
            proj_group(qT, "wq", "bq", 0, 0, on_act=True)
            project_v(0, 4)
            transpose_group(2)
            proj_group(kT, "wk", "bk", 0, 2, on_act=True)
            transpose_group(3)
            proj_group(kT, "wk", "bk", 0, 3, on_act=True)
            project_v(4, 16)
            for sg in (1, 2, 3):
                proj_group(qT, "wq", "bq", 0, sg)
            for sg in range(4):
                proj_group(kT, "wk", "bk", 1, sg)
            for sg in range(4):
                proj_group(qT, "wq", "bq", 1, sg)
            for jq in range(4):
                attend_pair_quarter(0, jq)
                attend_pair_quarter(1, jq)
                if jq > 0:
                    out_quarter(jq - 1)
            out_quarter(3)

    _split_excess_waits(nc)
    return nc


def _in_maps(inputs):
    x = np.ascontiguousarray(np.asarray(inputs["x"], dtype=np.float32))
    maps = []
    for c in range(N_CORES):
        b, g = c // 2, c % 2
        hs = slice(g * DQ, (g + 1) * DQ)
        maps.append({
            "x": np.ascontiguousarray(x[b].T),
            "wq": np.ascontiguousarray(np.asarray(inputs["Wq"], np.float32)[:, hs]),
            "wk": np.ascontiguousarray(np.asarray(inputs["Wk"], np.float32)[:, hs]),
            "wv": np.ascontiguousarray(np.asarray(inputs["Wv"], np.float32)[:, hs]),
            "wh": np.ascontiguousarray(np.asarray(inputs["Wh"], np.float32)[hs, :]),
            "bq": np.ascontiguousarray(np.asarray(inputs["bq"], np.float32)[hs]),
            "bk": np.ascontiguousarray(np.asarray(inputs["bk"], np.float32)[hs]),
            "bv": np.ascontiguousarray(np.asarray(inputs["bv"], np.float32)[hs]),
        })
    return maps


def kernel(**inputs):
    from concourse.bass_utils import run_bass_kernel_spmd

    nc = build_nc(nrep=1)
    maps = _in_maps(inputs)
    res = run_bass_kernel_spmd(nc, maps, core_ids=list(range(N_CORES)))
    bh = np.asarray(inputs["bh"], np.float32)
    out = np.empty((B, S, D_MODEL), np.float32)
    for b in range(B):
        out[b] = res.results[2 * b]["o"] + res.results[2 * b + 1]["o"] + bh
    return out

